# revision 24
# baseline (speedup 1.0000x reference)
"""Multi-head self-attention Trainium2 kernel (8 NeuronCores).

Problem: B=4, S=2048, D=1024, H=8 heads (HD=128).
  qkv = x @ qkv_w.T + qkv_b ; q,k,v = split(qkv)
  q = (q @ q_w.T + q_b)  (same k, v) -> [B,H,S,HD]
  scores = q k^T * HD^-0.5, masked softmax (attn_mask==1 -> -inf), o = attn @ v
  out = o @ out_w.T + out_b

Sharding: 8 cores = 4 batches x 2 head-groups (4 heads each).
Core c: batch b = c % 4, head-group g = c // 4.

Host-side algebraic folding: the qkv projection and per-stream q/k/v
projections are both linear, so they are composed into single effective
weights (W_eff = w @ qkv_w_slice), halving device matmul work. The
out-projection is row-parallel across head-groups; the two partial
outputs per batch are summed on host (the unshard step) with out_b.

Device flow per core (all matmuls bf16 with fp32 PSUM accumulation):
  qT_h[HD,S], kT_h[HD,S] = W x^T      (contraction over D on partitions)
  v[S, 4*HD]                          (natural layout)
  per head, per q-half (1024 q), software-pipelined 2 chunks deep:
    for kc in 16 k-chunks:
      sT = kT_h[:,kc]^T @ qT_h        [128 k, 1024 q]   (PE -> PSUM f32)
      p  = exp(SCALE * sT)            (ACT -> bf16 SBUF)
      pm = p * keepT[kc]              (DVE; keep = attn_mask.T == 0)
      oT += v[kc]^T-as-lhsT @ pm      -> oT[HD, q]      (PE, PSUM accum)
      acc(D or G) += pm               denominator chains on DVE / GpSimd
    fold: dB = ones^T @ (accD | accG) (2+2 small PE matmuls -> PSUM)
    rdb = reciprocal_approx_fast(dB)  (custom DVE op)
    oT_sb = oT * rdb                  (GpSimd, PSUM*SBUF -> bf16)
  out_partial[s,:] = sum_h oT_h[:,s_chunk]^T @ outwT_h   (+host bias/sum)
  The out-projection for q-half 0 is interleaved into q-half 1's
  attention; outputs DMA straight from PSUM.
"""

import os
import sys
import types

sys.path.insert(0, "/opt/trn_rl_repo")

import numpy as np
import ml_dtypes

BF16 = ml_dtypes.bfloat16

B, S, D, H, HD = 4, 2048, 1024, 8, 128
HG = 2           # head groups
HPG = H // HG    # heads per group (4)
GD = HPG * HD    # dims per group (512)
SCALE = float(HD) ** -0.5
NKC = S // 128   # 16 k chunks
NSC = S // 128   # 16 s chunks
ND = D // 128    # 8 d chunks

# --- tunables ---
INTERLEAVE = False       # interleave half-0 outproj into half-1 attention
RECIP_MAGIC = 0x7EF127EA  # f32 reciprocal bit-trick seed (then 1 Newton step)
# denominator routing per kc-pair: one GpSimd chain over pairs {0,2,4,6}
# (3 spread-out adds, no pm-pool gridlock), one DVE chain over {1,3}, and
# pairs {5,7} contracted directly by extra PE fold matmuls.
GPS_CHAIN_PAIRS = (0, 2, 4, 6)
DVE_CHAIN_PAIRS = (1, 3)
DIRECT_PAIRS = (5, 7)

_cached = {}


def _install_ntff_hook_shim():
    """The agent image's antenv lacks axon_hooks; shim it so trace works."""
    if "antenv.axon_hooks" in sys.modules:
        return
    try:
        import trn_agent_boot.trn_boot as _tb

        _hook = _tb._ntff_profile_via_ctypes("/opt/axon/libaxon_pjrt.so")
    except Exception:
        _hook = None
    _m = types.ModuleType("antenv.axon_hooks")
    _m.get_axon_ntff_profile_hook = lambda: _hook
    sys.modules["antenv.axon_hooks"] = _m


def _split_waits(nc, mybir, maxw=1):
    """Walrus in this image allows only one sync wait per instruction;
    hoist extra waits onto preceding NoOps on the same engine."""
    n_new = 0
    for fn in nc.m.functions:
        for bb in fn.blocks:
            newlist = []
            for inst in bb.instructions:
                si = inst.sync_info
                if si is not None and si.on_wait is not None and len(si.on_wait) > maxw:
                    waits = list(si.on_wait)
                    extra, keep = waits[:-maxw], waits[-maxw:]
                    while extra:
                        chunk, extra = extra[:maxw], extra[maxw:]
                        nop = mybir.InstNoOp(name=f"I-waitsplit-{nc.next_id()}")
                        nop.engine = inst.engine
                        nop.sync_info = mybir.SyncInfo(on_wait=chunk, on_update=[])
                        newlist.append(nop)
                        n_new += 1
                    si.on_wait = keep
                newlist.append(inst)
            bb.instructions = newlist
    return n_new


def _build_program(use_vbias=True):
    import concourse.bass as bass
    import concourse.mybir as mybir
    import concourse.tile as tile

    f32 = mybir.dt.float32
    bf16 = mybir.dt.bfloat16
    Exp = mybir.ActivationFunctionType.Exp
    Ident = mybir.ActivationFunctionType.Identity
    Ln = mybir.ActivationFunctionType.Ln
    Div = mybir.AluOpType.divide

    nc = bass.Bass()

    # DRAM parameters (per-core shards, pre-tiled on host)
    xT = nc.declare_dram_parameter("xT", [ND, 128, S], bf16, isOutput=False)
    wqT = nc.declare_dram_parameter("wqT", [128, ND * GD], bf16, isOutput=False)
    wkT = nc.declare_dram_parameter("wkT", [128, ND * GD], bf16, isOutput=False)
    wvT = nc.declare_dram_parameter("wvT", [128, ND * GD], bf16, isOutput=False)
    bq = nc.declare_dram_parameter("bq", [128, HPG], f32, isOutput=False)
    bk = nc.declare_dram_parameter("bk", [128, HPG], f32, isOutput=False)
    bvrow = nc.declare_dram_parameter("bvrow", [1, GD], bf16, isOutput=False)
    outwT = nc.declare_dram_parameter("outwT", [128, HPG * D], bf16, isOutput=False)
    keepT = nc.declare_dram_parameter("keepT", [NKC, 128, S], bf16, isOutput=False)
    out = nc.declare_dram_parameter("out", [S, D], f32, isOutput=True)

    with tile.TileContext(nc) as tc:
        import contextlib

        with contextlib.ExitStack() as ctx:
            # --- pools ---
            # xT and keepT share one 16-slot rotation of [128, S] bf16 tiles.
            p_big = ctx.enter_context(tc.tile_pool(name="big2k", bufs=16))
            p_pers = ctx.enter_context(tc.tile_pool(name="pers", bufs=1))
            p_pm = ctx.enter_context(tc.tile_pool(name="pm", bufs=4))
            p_acc = ctx.enter_context(tc.tile_pool(name="acc", bufs=2))
            p_sm = ctx.enter_context(tc.tile_pool(name="small", bufs=2))
            p_osb = ctx.enter_context(tc.tile_pool(name="osb", bufs=2))
            pp_sT = ctx.enter_context(tc.tile_pool(name="ppsT", bufs=2, space="PSUM"))
            pp_sm = ctx.enter_context(tc.tile_pool(name="ppsm", bufs=4, space="PSUM"))

            # --- constants ---
            ones128 = p_pers.tile([128, 128], bf16, tag="ones128", name="ones128")
            nc.vector.memset(ones128, 1.0)
            i32 = mybir.dt.int32
            magic = p_pers.tile([128, 512], i32, tag="magic", name="magic")
            nc.vector.memset(magic, RECIP_MAGIC)

            # --- input DMAs -------------------------------------------------
            # sync queue: projection-critical (wq, xT chunks, wk, wv, biases)
            # scalar queue: attention/outproj inputs (keepT, outw)
            w_sb = {}
            t = p_pers.tile([128, ND * GD], bf16, tag="wq", name="wq")
            nc.sync.dma_start(out=t, in_=wqT[:, :])
            w_sb["q"] = t
            xt_tiles = []
            for d in range(ND):
                t = p_big.tile([128, S], bf16, tag="big2k", name="big2k")
                nc.sync.dma_start(out=t, in_=xT[d])
                xt_tiles.append(t)
            for name, drm in (("k", wkT), ("v", wvT)):
                t = p_pers.tile([128, ND * GD], bf16, tag=f"w{name}", name=f"w{name}")
                nc.sync.dma_start(out=t, in_=drm[:, :])
                w_sb[name] = t

            bq_sb = p_pers.tile([128, HPG], f32, tag="bq", name="bq_sb")
            nc.sync.dma_start(out=bq_sb, in_=bq[:, :])
            bk_sb = p_pers.tile([128, HPG], f32, tag="bk", name="bk_sb")
            nc.sync.dma_start(out=bk_sb, in_=bk[:, :])
            bv_sb = None
            if use_vbias:
                bv_sb = p_pers.tile([1, GD], bf16, tag="bv", name="bv_sb")
                nc.sync.dma_start(out=bv_sb, in_=bvrow[:, :])

            keep_tiles = [None] * NKC
            for kc in range(8):
                t = p_big.tile([128, S], bf16, tag="big2k", name="big2k")
                nc.scalar.dma_start(out=t, in_=keepT[kc])
                keep_tiles[kc] = t
            outw_sb = p_pers.tile([128, HPG * D], bf16, tag="outw", name="outw")
            nc.scalar.dma_start(out=outw_sb, in_=outwT[:, :])

            def w_sl(name, d):
                return w_sb[name][:, d * GD:(d + 1) * GD]

            def xT_sl(d, lo, hi):
                return xt_tiles[d][:, lo:hi]

            def keep_sl(kc, lo, hi):
                return keep_tiles[kc][:, lo:hi]

            def outw_sl(h, nn):
                return outw_sb[:, h * D + nn * 512:h * D + (nn + 1) * 512]

            # --- projections (d-major, 4 concurrent PSUM accumulators so the
            # d=0 matmuls of a group start as soon as xT[0]/w land) ---
            qT_sb = [p_pers.tile([128, S], bf16, tag=f"qT{h}", name=f"qT{h}") for h in range(HPG)]
            kT_sb = [p_pers.tile([128, S], bf16, tag=f"kT{h}", name=f"kT{h}") for h in range(HPG)]

            units = []  # (stream, head, quarter)
            for name, dst, bias in (("q", qT_sb, bq_sb), ("k", kT_sb, bk_sb)):
                for h in range(HPG):
                    for qu in range(4):
                        units.append((name, dst, bias, h, qu))
            for gstart in range(0, len(units), 4):
                group = units[gstart:gstart + 4]
                pss = [
                    pp_sm.tile([128, 512], f32, tag="ppsm", name="ppsm")
                    for _ in group
                ]
                for d in range(ND):
                    for (name, dst, bias, h, qu), ps in zip(group, pss):
                        nc.tensor.matmul(
                            ps,
                            lhsT=w_sl(name, d)[:, h * 128:(h + 1) * 128],
                            rhs=xT_sl(d, qu * 512, (qu + 1) * 512),
                            start=(d == 0),
                            stop=(d == ND - 1),
                        )
                for (name, dst, bias, h, qu), ps in zip(group, pss):
                    nc.scalar.activation(
                        out=dst[h][:, qu * 512:(qu + 1) * 512],
                        in_=ps,
                        func=Ident,
                        bias=bias[:, h:h + 1],
                    )

            v_sb = [p_pers.tile([128, GD], bf16, tag=f"v{sc}", name=f"v{sc}") for sc in range(NSC)]
            for sc in range(NSC):
                ps = pp_sm.tile([128, GD], f32, tag="ppsm", name="ppsm")
                for d in range(ND):
                    nc.tensor.matmul(
                        ps,
                        lhsT=xT_sl(d, sc * 128, (sc + 1) * 128),
                        rhs=w_sl("v", d),
                        start=(d == 0),
                        stop=(d == ND - 1) and not use_vbias,
                    )
                if use_vbias:
                    # bias via K=1 ones row
                    nc.tensor.matmul(
                        ps,
                        lhsT=ones128[0:1, :],
                        rhs=bv_sb,
                        start=False,
                        stop=True,
                    )
                nc.vector.tensor_copy(v_sb[sc], ps)

            # --- second half of keepT (reuses xT slots once proj done) ---
            for kc in range(8, NKC):
                t = p_big.tile([128, S], bf16, tag="big2k", name="big2k")
                nc.scalar.dma_start(out=t, in_=keepT[kc])
                keep_tiles[kc] = t

            # --- attention + interleaved output projection ---
            oT_sb = [p_pers.tile([128, S], bf16, tag=f"oT{h}", name=f"oT{h}") for h in range(HPG)]

            def outproj_piece(sc, nn):
                # partial out[sc, nn-half] = sum_h oT_h[:, sc]^T @ outw_h
                ps = pp_sm.tile([128, 512], f32, tag="ppsm", name="ppsm")
                for hx in range(HPG):
                    nc.tensor.matmul(
                        ps,
                        lhsT=oT_sb[hx][:, sc * 128:(sc + 1) * 128],
                        rhs=outw_sl(hx, nn),
                        start=(hx == 0),
                        stop=(hx == HPG - 1),
                    )
                dst = out[sc * 128:(sc + 1) * 128, nn * 512:(nn + 1) * 512]
                osb = p_osb.tile([128, 1024], f32, tag="osb", name="osb")
                nc.vector.tensor_copy(osb[:, 0:512], ps)
                nc.sync.dma_start(out=dst, in_=osb[:, 0:512])

            for half in range(2):
                # pieces of the PREVIOUS half's outproj to interleave
                pieces = []
                if INTERLEAVE and half == 1:
                    pieces = [(sc, nn) for sc in range(8) for nn in range(2)]
                pi = 0

                for h in range(HPG):
                    q0 = half * 1024
                    o_ps = [pp_sm.tile([128, 512], f32, tag="ppsm", name="ppsm") for _ in range(2)]
                    # denominator chain accumulators; [:, 0:1024] sums the
                    # even chunk of each pair, [:, 1024:] the odd chunk
                    accG = p_acc.tile([128, 2048], bf16, tag="acc", name="accG")
                    accD = p_acc.tile([128, 2048], bf16, tag="acc", name="accD")
                    chain_first = {}
                    direct_pms = {}

                    def consume(kcp, pm2):
                        # oT accumulation for pair kcp, issued one pair late
                        # so the PE never waits on ACT/DVE for this pair
                        for sub in range(2):
                            kc = 2 * kcp + sub
                            for qq in range(2):
                                nc.tensor.matmul(
                                    o_ps[qq],
                                    lhsT=v_sb[kc][:, h * 128:(h + 1) * 128],
                                    rhs=pm2[:, sub * 1024 + qq * 512:sub * 1024 + (qq + 1) * 512],
                                    start=(kc == 0),
                                    stop=(kc == NKC - 1),
                                )

                    pending = []  # [(kcp, pm2)] — 1-pair delay
                    for kcp in range(NKC // 2):
                        pm2 = p_pm.tile([128, 2048], bf16, tag="pm", name="pm")
                        for sub in range(2):
                            kc = 2 * kcp + sub
                            sT = pp_sT.tile([128, 1024], f32, tag="ppsT", name="ppsT")
                            for nn in range(2):
                                nc.tensor.matmul(
                                    sT[:, nn * 512:(nn + 1) * 512],
                                    lhsT=kT_sb[h][:, kc * 128:(kc + 1) * 128],
                                    rhs=qT_sb[h][:, q0 + nn * 512:q0 + (nn + 1) * 512],
                                    start=True,
                                    stop=True,
                                )
                            nc.scalar.activation(
                                out=pm2[:, sub * 1024:(sub + 1) * 1024],
                                in_=sT, func=Exp, scale=SCALE,
                            )
                            # mask multiply in place (bf16, SBUF, 2x mode)
                            nc.vector.tensor_mul(
                                pm2[:, sub * 1024:(sub + 1) * 1024],
                                pm2[:, sub * 1024:(sub + 1) * 1024],
                                keep_sl(kc, q0, q0 + 1024),
                            )
                        # denominator routing (see module-level comment)
                        if kcp in GPS_CHAIN_PAIRS:
                            ck, eng, acc = "G", nc.gpsimd, accG
                            idx = GPS_CHAIN_PAIRS.index(kcp)
                        elif kcp in DVE_CHAIN_PAIRS:
                            ck, eng, acc = "D", nc.vector, accD
                            idx = DVE_CHAIN_PAIRS.index(kcp)
                        else:
                            ck = None  # direct-folded by PE at the end
                            direct_pms[kcp] = pm2
                        if ck is not None:
                            if idx == 0:
                                chain_first[ck] = pm2
                            elif idx == 1:
                                eng.tensor_add(acc, chain_first[ck], pm2)
                            else:
                                eng.tensor_add(acc, acc, pm2)

                        pending.append((kcp, pm2))
                        if len(pending) > 1:
                            consume(*pending.pop(0))
                        # interleave previous half's outproj pieces
                        if pieces and kcp % 2 == 1 and pi < len(pieces):
                            outproj_piece(*pieces[pi])
                            pi += 1
                    for item in pending:
                        consume(*item)

                    for qq in range(2):
                        fold = pp_sm.tile([128, 512], f32, tag="ppsm", name="ppsm")
                        srcs = []
                        for t2 in [accG, accD] + [direct_pms[kp] for kp in DIRECT_PAIRS]:
                            srcs.append(t2[:, qq * 512:(qq + 1) * 512])
                            srcs.append(t2[:, 1024 + qq * 512:1024 + (qq + 1) * 512])
                        for si, src in enumerate(srcs):
                            nc.tensor.matmul(
                                fold,
                                lhsT=ones128,
                                rhs=src,
                                start=(si == 0),
                                stop=(si == len(srcs) - 1),
                            )
                        # 1/dB via bit-trick seed + one Newton step, DVE-only.
                        # y0 = bits(magic - bits(d)); t = d*y0;
                        # rdbn = (t-2)*y0 = -1/d (+O(0.2%)) — the sign flip
                        # is absorbed by the host-side unshard negation.
                        y0 = p_sm.tile([128, 512], f32, tag="y0", name="y0")
                        nc.vector.tensor_tensor(
                            y0.bitcast(i32), magic, fold.bitcast(i32),
                            mybir.AluOpType.subtract,
                        )
                        rdbn = p_sm.tile([128, 512], f32, tag="rdbn", name="rdbn")
                        nc.vector.tensor_mul(rdbn, fold, y0)
                        nc.vector.scalar_tensor_tensor(
                            rdbn, rdbn, 2.0, y0,
                            mybir.AluOpType.subtract, mybir.AluOpType.mult,
                        )
                        dst = oT_sb[h][:, q0 + qq * 512:q0 + (qq + 1) * 512]
                        nc.vector.tensor_mul(dst, o_ps[qq], rdbn)

            # --- tail: output projection for the last half (and, when not
            # interleaving, the first). sT PSUM tiles are free here; DVE is
            # idle so the f32 copies are off the critical path. ---
            tail_scs = list(range(8, NSC)) + ([] if INTERLEAVE else list(range(8)))
            for sc in tail_scs:
                ps = pp_sT.tile([128, 1024], f32, tag="ppsT", name="ppsT")
                for hx in range(HPG):
                    for nn in range(2):
                        nc.tensor.matmul(
                            ps[:, nn * 512:(nn + 1) * 512],
                            lhsT=oT_sb[hx][:, sc * 128:(sc + 1) * 128],
                            rhs=outw_sl(hx, nn),
                            start=(hx == 0),
                            stop=(hx == HPG - 1),
                        )
                osb = p_osb.tile([128, 1024], f32, tag="osb", name="osb")
                nc.vector.tensor_copy(osb, ps)
                nc.sync.dma_start(out=out[sc * 128:(sc + 1) * 128, :], in_=osb)

    _split_waits(nc, mybir, maxw=1)
    return nc


def _prep_core_inputs(x, attn_mask, qkv_w, qkv_b, q_w, q_b, k_w, k_b, v_w, v_b,
                      out_w):
    """Host-side: fold projections, shard, pre-transpose/tile, cast."""
    f = np.float32
    x = np.asarray(x, f)
    qkv_w = np.asarray(qkv_w, f)
    qkv_b = np.asarray(qkv_b, f)
    Ws = {}
    bs = {}
    for i, (w, b) in enumerate(((q_w, q_b), (k_w, k_b), (v_w, v_b))):
        w = np.asarray(w, f)
        b = np.asarray(b, f)
        sl = slice(i * D, (i + 1) * D)
        Ws[i] = w @ qkv_w[sl]              # [D, D] effective
        bs[i] = b + w @ qkv_b[sl]          # [D]
    out_wT = np.ascontiguousarray(np.asarray(out_w, f).T)  # [D(hd), D(model)]

    keepT = (np.asarray(attn_mask).T == 0).astype(BF16)    # [k, q]
    keepT_t = np.ascontiguousarray(keepT).reshape(NKC, 128, S)

    xT_all = []
    for b_i in range(B):
        xb = np.ascontiguousarray(x[b_i].T.astype(BF16))   # [D, S]
        xT_all.append(xb.reshape(ND, 128, S))

    def dmajor(wT):
        # [D, GD] -> [128, ND*GD] with d-chunk-major columns
        return np.ascontiguousarray(
            wT.reshape(ND, 128, -1).transpose(1, 0, 2).reshape(128, -1)
        )

    maps = []
    for c in range(8):
        b_i = c % B
        g = c // B
        sl = slice(g * GD, (g + 1) * GD)
        m = {
            "xT": xT_all[b_i],
            "wqT": dmajor(Ws[0][sl].T.astype(BF16)),
            "wkT": dmajor(Ws[1][sl].T.astype(BF16)),
            "wvT": dmajor(Ws[2][sl].T.astype(BF16)),
            "bq": np.ascontiguousarray(bs[0][sl].reshape(HPG, 128).T.astype(f)),
            "bk": np.ascontiguousarray(bs[1][sl].reshape(HPG, 128).T.astype(f)),
            "bvrow": bs[2][sl].astype(BF16).reshape(1, GD),
            "outwT": np.ascontiguousarray(
                out_wT[sl].astype(BF16).reshape(HPG, 128, D)
                .transpose(1, 0, 2).reshape(128, HPG * D)
            ),
            "keepT": keepT_t,
        }
        maps.append(m)
    return maps


def kernel(x, attn_mask, qkv_w, qkv_b, q_w, q_b, k_w, k_b, v_w, v_b,
           out_w, out_b, _trace=False):
    _install_ntff_hook_shim()
    from concourse.bass_utils import run_bass_kernel_spmd

    in_maps = _prep_core_inputs(
        x, attn_mask, qkv_w, qkv_b, q_w, q_b, k_w, k_b, v_w, v_b, out_w
    )
    use_vbias = bool(np.any(np.asarray(in_maps[0]["bvrow"], np.float32) != 0))
    key = ("nc", use_vbias)
    if key not in _cached:
        _cached[key] = _build_program(use_vbias=use_vbias)
    nc = _cached[key]
    core_ids = list(range(8))
    try:
        res = run_bass_kernel_spmd(nc, in_maps, core_ids, trace=_trace)
    except Exception:
        # transient NRT device wedge recovers on retry
        res = run_bass_kernel_spmd(nc, in_maps, core_ids, trace=_trace)
    _cached["last_result"] = res

    out_b = np.asarray(out_b, np.float32)
    full = np.empty((B, S, D), np.float32)
    for b_i in range(B):
        # device partials carry a negated sign (Newton-reciprocal trick)
        full[b_i] = (
            out_b - res.results[b_i]["out"] - res.results[b_i + B]["out"]
        )
    return full
